# revision 1
# baseline (speedup 1.0000x reference)
"""Trainium2 Bass kernel for an AttentionBlock (GroupNorm + 8-head self-attn + proj + residual).

Sharding: data-parallel over batch. B=8 batch elements -> one per NeuronCore.
Each core runs an identical program on its own [C=512, T=1024] slice; the host
shards inputs / gathers outputs. No collectives.

Per-core pipeline (layouts partition-major, 128 partitions):
  x      [128p, 4ct, 1024t]   channels on partitions (fp32, kept for residual)
  GroupNorm stats: free-dim sum on VectorE + Square-with-accumulator on
     ScalarE; 16-channel group reduction and mean/rstd broadcast across
     partitions via tiny fp32 PE matmuls with 0/1 group matrices;
     rstd = exp(-0.5*ln(var+eps)); affine applied on ScalarE -> xn (bf16)
  qk   = Wqk @ xn + b  (bf16 matmuls, fp32 psum)  rows = [q(512) | k(512)],
         head-major 64-row blocks
  vT   = xn^T @ WvT    (bf16) stored per head-pair as [v_2j |1|1| v_2j+1]
         128-col blocks (ones columns make the AV matmul emit the softmax
         denominator Z replicated on the opposite partition half)
  lT[s,t] = k_h^T q_h per head: K=64 matmuls row-packed 2 heads per PE pass
  w    = exp(lT/8) on ScalarE, no max-subtraction (logits/8 in [-7,8]) -> bf16
  a|Z  = [v|1]^T @ w : 8-deep psum accumulation; per head the attention rows
         land on their final partition half, Z on the other
  1/Z  via DVE reciprocal_approx_fast at base partition 0 (custom-DVE ucode
         breaks at other bases); a 64-partition SBUF->SBUF DMA does the
         cross-partition move (engines cannot shift partitions)
  out  = WpT^T @ a * (1/Z) ... + (proj_b + Wp @ v_bias) + x  (v-bias folded on
         host; bias-add on ScalarE, residual on VectorE)

Schedule: one-pair software pipeline -- logits/exp of pair j+1 are emitted
before AV of pair j, with the remaining qk row-tiles spread across the pair
windows to keep the PE busy (HAM clock-throttle avoidance).
"""

import numpy as np

import concourse.bass as bass
import concourse.mybir as mybir
import concourse.tile as tile
from concourse import bacc
from contextlib import ExitStack

B = 8
C = 512
T = 1024
NH = 8            # heads
CH = 64           # channels per head
G = 32            # groups
CPG = C // G      # 16 channels per group
EPS = 1e-5
P = 128
NCT = C // P      # 4 channel tiles
NTT = T // P      # 8 sequence tiles
NQK = (2 * C) // P  # 8 row tiles of [q;k]
NC2 = T // 512    # 2 free-dim chunks of 512
HS = 2 * CH       # 128: per-head lhsT width in vT (64 v cols + 64 ones cols)

F32 = mybir.dt.float32
F32R = mybir.dt.float32r
BF16 = mybir.dt.bfloat16
FP8 = mybir.dt.float8e4
FX = mybir.ActivationFunctionType
ALU = mybir.AluOpType
AX = mybir.AxisListType


_DEBUG_TAP = None  # set by debug scripts before build_nc()
_STAGE = 5  # 1=GN/xn 2=+qkv/vT 3=+logits/exp 4=+AV 5=full (timing bisection)


def _build_body(ctx, tc, io):
    nc = tc.nc
    x_d = io["x"]
    out_d = io["out"]

    pers = ctx.enter_context(tc.tile_pool(name="pers", bufs=1))
    wt_pool = ctx.enter_context(tc.tile_pool(name="wt", bufs=3))
    small = ctx.enter_context(tc.tile_pool(name="small", bufs=2))
    outp = ctx.enter_context(tc.tile_pool(name="outp", bufs=3))
    ps_a = ctx.enter_context(tc.tile_pool(name="ps_a", bufs=2, space="PSUM"))
    ps_b = ctx.enter_context(tc.tile_pool(name="ps_b", bufs=4, space="PSUM"))

    # ---- persistent SBUF tensors -------------------------------------------
    x_sb = pers.tile([P, NCT, T], F32, tag="x")
    xn_sb = pers.tile([P, NCT, T], BF16, tag="xn")
    wqkT_sb = pers.tile([P, NCT, 2 * C], BF16, tag="wqkT")
    wvT_sb = pers.tile([P, NCT, C], BF16, tag="wvT")
    wpT_sb = pers.tile([P, NCT, C], BF16, tag="wpT")
    qk_sb = pers.tile([P, NQK, T], BF16, tag="qk")
    vT_sb = pers.tile([P, NTT, NH * HS], BF16, tag="vT")
    a_sb = pers.tile([P, NCT, T], BF16, tag="a")
    gw_sb = pers.tile([P, NCT], F32, tag="gw")
    gb_sb = pers.tile([P, NCT], F32, tag="gb")
    bqk_sb = pers.tile([P, NQK], F32, tag="bqk")
    bp_sb = pers.tile([P, NCT], F32, tag="bp")
    gmat_sb = pers.tile([P, NCT, G], F32, tag="gmat")
    gtmat_sb = pers.tile([G, NCT, P], F32, tag="gtmat")
    stats_sb = pers.tile([P, NCT, 2], F32, tag="stats")
    stats2_sb = pers.tile([G, 2], F32, tag="stats2")
    ab_sb = pers.tile([P, NCT, 2], F32, tag="ab")
    xsq_sb = pers.tile([P, T], F32, tag="xsq")

    # ---- input DMAs ---------------------------------------------------------
    # x split per channel-tile (4 DMA queues in parallel; GN stats for tile j
    # start as soon as slice j lands)
    for j in range(NCT):
        nc.sync.dma_start(x_sb[:, j, :], x_d[j])
    nc.sync.dma_start(gmat_sb[:], io["gmat"])
    nc.sync.dma_start(gtmat_sb[:], io["gtmat"])
    nc.sync.dma_start(gw_sb[:], io["gw"])
    nc.sync.dma_start(gb_sb[:], io["gb"])
    nc.sync.dma_start(wqkT_sb[:], io["wqkT"])
    nc.sync.dma_start(bqk_sb[:], io["bqk"])
    nc.sync.dma_start(wvT_sb[:], io["wvT"])
    nc.sync.dma_start(wpT_sb[:], io["wpT"])
    nc.sync.dma_start(bp_sb[:], io["bp"])

    # Per head-pair j, vT cols j*256..(j+1)*256 hold [v_2j | ones | ones | v_2j+1].
    # The AV lhsT for head h is cols h*128..(h+1)*128: [v|1] for even heads,
    # [1|v] for odd heads, so each head's attention rows land on the same
    # partition half as its final slot in a_sb, and the softmax denominator
    # lands replicated on the other half -- every consumer op stays
    # partition-aligned (HW engines cannot shift data across partitions).
    ones_view = vT_sb[:, :, : NH * HS].rearrange(
        "p s (pr i) -> p s pr i", i=2 * HS
    )[:, :, :, CH : CH + HS]
    nc.vector.memset(ones_view, 1.0)

    # ---- GroupNorm statistics ----------------------------------------------
    # Per-channel sum and sum-of-squares on ScalarE: the activation's fused
    # accumulator reduces along the free dim for free, and ScalarE is
    # otherwise idle during the prologue.
    for j in range(NCT):
        nc.vector.reduce_sum(stats_sb[:, j, 0:1], x_sb[:, j, :], axis=AX.X)
        nc.scalar.activation(
            xsq_sb[:], x_sb[:, j, :], FX.Square, accum_out=stats_sb[:, j, 1:2]
        )
    gstat_ps = ps_b.tile([P, 512], F32, tag="psb")
    for j in range(NCT):
        nc.tensor.matmul(
            gstat_ps[0:G, 0:2],
            lhsT=gmat_sb[:, j, :],
            rhs=stats_sb[:, j, :],
            start=(j == 0),
            stop=(j == NCT - 1),
        )
    # finalize on 32 partitions: mean, var -> rstd (sqrt + recip + 1 Newton step)
    mv = small.tile([G, 2], F32, tag="mv")
    nc.vector.tensor_scalar_mul(mv[:], gstat_ps[0:G, 0:2], 1.0 / (CPG * T))
    msq = small.tile([G, 1], F32, tag="msq")
    nc.vector.tensor_mul(msq[:], mv[:, 0:1], mv[:, 0:1])
    veps = small.tile([G, 1], F32, tag="veps")
    nc.vector.tensor_scalar(veps[:], msq[:], -1.0, EPS, ALU.mult, ALU.add)
    nc.vector.tensor_tensor(veps[:], mv[:, 1:2], veps[:], ALU.add)  # E2-mean^2+eps
    # rstd = exp(-0.5 * ln(var+eps)); Ln and Exp share one ACT table set and
    # are far more accurate than the Sqrt table (~2 ULP vs 65536 budget)
    lnv = small.tile([G, 1], F32, tag="lnv")
    nc.scalar.activation(lnv[:], veps[:], FX.Ln)
    nc.scalar.activation(stats2_sb[:, 1:2], lnv[:], FX.Exp, scale=-0.5)
    nc.vector.tensor_copy(stats2_sb[:, 0:1], mv[:, 0:1])

    # broadcast (mean, rstd) to channels; A = w*rstd, B = b - mean*A; xn = A*x+B
    ab_ps = ps_b.tile([P, 512], F32, tag="psb")
    for j in range(NCT):
        nc.tensor.matmul(
            ab_ps[:, 2 * j : 2 * j + 2],
            lhsT=gtmat_sb[:, j, :],
            rhs=stats2_sb[:],
            start=True,
            stop=True,
            skip_group_check=True,
        )
    mean_v = ab_ps[:, 0:8].rearrange("p (j two) -> p j two", two=2)[:, :, 0]
    rstd_v = ab_ps[:, 0:8].rearrange("p (j two) -> p j two", two=2)[:, :, 1]
    a_v = ab_sb[:, :, 0]
    b_v = ab_sb[:, :, 1]
    nc.vector.tensor_tensor(a_v, gw_sb[:, :], rstd_v, ALU.mult)
    nc.vector.tensor_tensor(b_v, mean_v, a_v, ALU.mult)
    nc.vector.tensor_tensor(b_v, gb_sb[:, :], b_v, ALU.subtract)
    for j in range(NCT):
        nc.scalar.activation(
            xn_sb[:, j, :],
            x_sb[:, j, :],
            FX.Identity,
            bias=ab_sb[:, j, 1:2],
            scale=ab_sb[:, j, 0:1],
        )

    # ---- qk = Wqk @ xn + b --------------------------------------------------
    def emit_qk(m):
        for n in range(NC2):
            qk_ps = ps_b.tile([P, 512], F32, tag="psb")
            for k in range(NCT):
                nc.tensor.matmul(
                    qk_ps[:],
                    lhsT=wqkT_sb[:, k, m * P : (m + 1) * P],
                    rhs=xn_sb[:, k, n * 512 : (n + 1) * 512],
                    start=(k == 0),
                    stop=(k == NCT - 1),
                )
            nc.vector.tensor_scalar_add(
                qk_sb[:, m, n * 512 : (n + 1) * 512], qk_ps[:], bqk_sb[:, m : m + 1]
            )

    def emit_vt(st):
        vt_ps = ps_b.tile([P, 512], F32, tag="psb")
        for k in range(NCT):
            nc.tensor.matmul(
                vt_ps[:],
                lhsT=xn_sb[:, k, st * P : (st + 1) * P],
                rhs=wvT_sb[:, k, :],
                start=(k == 0),
                stop=(k == NCT - 1),
            )
        blk = vT_sb[:, st, : NH * HS].rearrange("p (pr i) -> p pr i", i=2 * HS)
        src = vt_ps[:].rearrange("p (pr half i) -> p pr half i", half=2, i=CH)
        nc.vector.tensor_copy(blk[:, :, 0:CH], src[:, :, 0, :])
        nc.vector.tensor_copy(blk[:, :, 2 * HS - CH : 2 * HS], src[:, :, 1, :])

    def emit_pair_logits(j, wt):
        """Logits + exp for head pair (2j, 2j+1); row-packed K=64 matmuls.
        Both halves of an s-tile share one 4-bank psum tile so the exp is a
        single FD=2048 ScalarE instruction (amortizes the per-op init)."""
        for st in range(NTT):
            for half in range(2):
                lt = ps_a.tile([P, T], F32, tag="psa")
                rs = slice(half * CH, (half + 1) * CH)
                for n in range(NC2):
                    nc.tensor.matmul(
                        lt[:, n * 512 : (n + 1) * 512],
                        lhsT=qk_sb[rs, 4 + j, st * P : (st + 1) * P],
                        rhs=qk_sb[rs, j, n * 512 : (n + 1) * 512],
                        start=True,
                        stop=True,
                    )
                nc.scalar.activation(
                    wt[:, st, half * T : (half + 1) * T], lt[:], FX.Exp, scale=0.125
                )

    def emit_pair_av(j, wt):
        for half in range(2):
            h = 2 * j + half
            for n in range(NC2):
                av_ps = ps_b.tile([P, 512], F32, tag="psb")
                for st in range(NTT):
                    nc.tensor.matmul(
                        av_ps[:],
                        lhsT=vT_sb[:, st, h * HS : (h + 1) * HS],
                        rhs=wt[:, st, half * T + n * 512 : half * T + (n + 1) * 512],
                        start=(st == 0),
                        stop=(st == NTT - 1),
                    )
                if _DEBUG_TAP == "av0" and j == 0:
                    dt_t = outp.tile([P, 512], F32, tag="dbg", name=f"dbg{h}_{n}")
                    nc.vector.tensor_copy(dt_t[:], av_ps[:])
                    nc.sync.dma_start(
                        out_d[:, half, n * 512 : (n + 1) * 512], dt_t[:]
                    )
                # even head: rows 0..63 = attention, 64..127 = Z (odd: swapped).
                # The custom-DVE reciprocal only works at base partition 0, and
                # engines can't shift data across partitions -- a small
                # SBUF->SBUF DMA does the 64-partition move.
                lo, hi = slice(0, CH), slice(CH, P)
                zs = small.tile([P, 512], F32, tag="zs")
                if half == 0:
                    zc = small.tile([P, 512], F32, tag="zc")
                    nc.vector.tensor_copy(zc[hi, :], av_ps[hi, :])
                    nc.sync.dma_start(zs[lo, :], zc[hi, :])
                    zi = small.tile([P, 512], F32, tag="zi")
                    nc.vector.reciprocal_approx_fast(zi[lo, :], zs[lo, :])
                    nc.vector.tensor_tensor(
                        a_sb[lo, j, n * 512 : (n + 1) * 512],
                        av_ps[lo, :],
                        zi[lo, :],
                        ALU.mult,
                    )
                else:
                    zi = small.tile([P, 512], F32, tag="zi")
                    nc.vector.reciprocal_approx_fast(zi[lo, :], av_ps[lo, :])
                    nc.sync.dma_start(zs[hi, :], zi[lo, :])
                    nc.vector.tensor_tensor(
                        a_sb[hi, j, n * 512 : (n + 1) * 512],
                        av_ps[hi, :],
                        zs[hi, :],
                        ALU.mult,
                    )

    def tap_out(src3d):
        for m in range(NCT):
            dt_t = outp.tile([P, T], F32, tag="dbg", name=f"stg{m}")
            nc.vector.tensor_copy(dt_t[:], src3d[:, m, :])
            nc.sync.dma_start(out_d[:, m, :], dt_t[:])

    if _STAGE == 1:
        tap_out(xn_sb)
        return

    # start attention on pair 0 as soon as its q/k row tiles exist
    emit_qk(0)
    emit_qk(4)
    wts0 = wt_pool.tile([P, NTT, 2 * T], BF16, tag="w", name="wt0")
    emit_pair_logits(0, wts0)
    if _DEBUG_TAP == "w0":
        for m in range(NCT):
            dt_t = outp.tile([P, T], F32, tag="dbg", name=f"dbg{m}")
            nc.vector.tensor_copy(dt_t[:], wts0[:, m, 0:T])
            nc.sync.dma_start(out_d[:, m, :], dt_t[:])
        return
    for st in range(NTT):
        emit_vt(st)
    if _DEBUG_TAP == "av0":
        emit_pair_av(0, wts0)
        return
    if _STAGE == 2:
        tap_out(qk_sb[:, 0:NCT, :])
        return
    if _STAGE == 3:
        # logits+exp only: tiny reads release the wt slots without AV
        dmy = small.tile([P, 8], F32, tag="dmy")
        nc.vector.tensor_copy(dmy[:, 0:8], wts0[:, 7, 2040:2048])
        for j in range(1, 4):
            wts = wt_pool.tile([P, NTT, 2 * T], BF16, tag="w", name=f"wt{j}")
            emit_pair_logits(j, wts)
            nc.vector.tensor_copy(dmy[:, 0:8], wts[:, 7, 2040:2048])
        tap_out(qk_sb[:, 0:NCT, :])
        return
    # One-pair software pipeline: logits(j+1) are emitted (= higher scheduler
    # priority) before AV(j), so ScalarE always has exp input ready while the
    # PE drains the previous pair's AV matmuls in the gaps. The remaining
    # qk row tiles are spread across the pair windows to keep the PE from
    # idling long enough to trip the HAM clock throttle.
    prev = wts0
    emit_qk(1)
    emit_qk(5)
    for j in range(1, 4):
        wts = wt_pool.tile([P, NTT, 2 * T], BF16, tag="w", name=f"wt{j}")
        emit_pair_logits(j, wts)
        emit_pair_av(j - 1, prev)
        if j < 3:
            emit_qk(j + 1)
            emit_qk(5 + j)
        prev = wts
    emit_pair_av(3, prev)
    if _STAGE == 4:
        tap_out(a_sb)
        return

    # optional debug tap: overwrite `out` with an intermediate tensor
    if _DEBUG_TAP == "xn":
        for m in range(NCT):
            dt_t = outp.tile([P, T], F32, tag="dbg", name=f"dbg{m}")
            nc.vector.tensor_copy(dt_t[:], xn_sb[:, m, :])
            nc.sync.dma_start(out_d[:, m, :], dt_t[:])
        return
    if _DEBUG_TAP == "qk":  # q rows: qk tiles 0..3 -> out tiles 0..3
        for m in range(NCT):
            dt_t = outp.tile([P, T], F32, tag="dbg", name=f"dbg{m}")
            nc.vector.tensor_copy(dt_t[:], qk_sb[:, m, :])
            nc.sync.dma_start(out_d[:, m, :], dt_t[:])
        return
    if _DEBUG_TAP == "k":  # k rows: qk tiles 4..7
        for m in range(NCT):
            dt_t = outp.tile([P, T], F32, tag="dbg", name=f"dbg{m}")
            nc.vector.tensor_copy(dt_t[:], qk_sb[:, 4 + m, :])
            nc.sync.dma_start(out_d[:, m, :], dt_t[:])
        return
    if _DEBUG_TAP == "vt":  # vT tiles 0..3 (with ones cols)
        for m in range(NCT):
            dt_t = outp.tile([P, T], F32, tag="dbg", name=f"dbg{m}")
            nc.vector.tensor_copy(dt_t[:], vT_sb[:, m, :])
            nc.sync.dma_start(out_d[:, m, :], dt_t[:])
        return
    if _DEBUG_TAP == "a":
        for m in range(NCT):
            dt_t = outp.tile([P, T], F32, tag="dbg", name=f"dbg{m}")
            nc.vector.tensor_copy(dt_t[:], a_sb[:, m, :])
            nc.sync.dma_start(out_d[:, m, :], dt_t[:])
        return

    # ---- proj + bias + residual --------------------------------------------
    for m in range(NCT):
        for n in range(NC2):
            pr_ps = ps_b.tile([P, 512], F32, tag="psb")
            for k in range(NCT):
                nc.tensor.matmul(
                    pr_ps[:],
                    lhsT=wpT_sb[:, k, m * P : (m + 1) * P],
                    rhs=a_sb[:, k, n * 512 : (n + 1) * 512],
                    start=(k == 0),
                    stop=(k == NCT - 1),
                )
            ot = outp.tile([P, 512], F32, tag="ot")
            # bias-add on ScalarE (idle in the tail), residual on VectorE
            nc.scalar.activation(
                ot[:], pr_ps[:], FX.Identity, bias=bp_sb[:, m : m + 1]
            )
            nc.vector.tensor_add(
                ot[:], ot[:], x_sb[:, m, n * 512 : (n + 1) * 512]
            )
            nc.sync.dma_start(out_d[:, m, n * 512 : (n + 1) * 512], ot[:])


def build_nc(loop_n=0):
    """loop_n > 0 wraps the body in a For_i running it loop_n times --
    used only by the timing harness to amortize host/RPC overhead."""
    nc = bacc.Bacc("TRN2", target_bir_lowering=False, debug=False)
    io = {}
    io["x"] = nc.dram_tensor("x", [NCT, P, T], F32, kind="ExternalInput").ap()
    io["gw"] = nc.dram_tensor("gw", [P, NCT], F32, kind="ExternalInput").ap()
    io["gb"] = nc.dram_tensor("gb", [P, NCT], F32, kind="ExternalInput").ap()
    io["wqkT"] = nc.dram_tensor("wqkT", [P, NCT, 2 * C], BF16, kind="ExternalInput").ap()
    io["bqk"] = nc.dram_tensor("bqk", [P, NQK], F32, kind="ExternalInput").ap()
    io["wvT"] = nc.dram_tensor("wvT", [P, NCT, C], BF16, kind="ExternalInput").ap()
    io["wpT"] = nc.dram_tensor("wpT", [P, NCT, C], BF16, kind="ExternalInput").ap()
    io["bp"] = nc.dram_tensor("bp", [P, NCT], F32, kind="ExternalInput").ap()
    io["gmat"] = nc.dram_tensor("gmat", [P, NCT, G], F32, kind="ExternalInput").ap()
    io["gtmat"] = nc.dram_tensor("gtmat", [G, NCT, P], F32, kind="ExternalInput").ap()
    io["out"] = nc.dram_tensor("out", [P, NCT, T], F32, kind="ExternalOutput").ap()
    with tile.TileContext(nc) as tc:
        with ExitStack() as ctx:
            if loop_n:
                with tc.For_i(0, loop_n, 1):
                    _build_body(ctx, tc, io)
            else:
                _build_body(ctx, tc, io)
    nc.compile()
    return nc


def _tile_cmaj(a, ntiles):
    """[ntiles*128, F...] -> [128, ntiles, F...] (partition-major tiling)."""
    return np.ascontiguousarray(
        a.reshape(ntiles, P, *a.shape[1:]).swapaxes(0, 1)
    )


def prep_inputs(x, norm_w, norm_b, qkv_w, qkv_b, proj_w, proj_b):
    f = np.float32
    x = np.asarray(x, f)
    norm_w = np.asarray(norm_w, f)
    norm_b = np.asarray(norm_b, f)
    qkv_w = np.asarray(qkv_w, f)
    qkv_b = np.asarray(qkv_b, f)
    proj_w = np.asarray(proj_w, f)
    proj_b = np.asarray(proj_b, f)

    wr = qkv_w.reshape(NH, 3, CH, C)
    Wq = wr[:, 0].reshape(C, C)
    Wk = wr[:, 1].reshape(C, C)
    Wv = wr[:, 2].reshape(C, C)
    br = qkv_b.reshape(NH, 3, CH)
    bq = br[:, 0].reshape(C)
    bk = br[:, 1].reshape(C)
    bv = br[:, 2].reshape(C)

    common = {}
    common["gw"] = _tile_cmaj(norm_w, NCT)
    common["gb"] = _tile_cmaj(norm_b, NCT)
    import ml_dtypes
    bf = ml_dtypes.bfloat16
    common["wqkT"] = _tile_cmaj(np.concatenate([Wq, Wk], 0).T.copy(), NCT).astype(bf)
    common["bqk"] = _tile_cmaj(np.concatenate([bq, bk]), NQK)
    common["wvT"] = _tile_cmaj(Wv.T.copy(), NCT).astype(bf)
    common["wpT"] = _tile_cmaj(proj_w.T.copy(), NCT).astype(bf)
    common["bp"] = _tile_cmaj(proj_b + proj_w @ bv, NCT)

    pidx = np.arange(P)
    gmat = np.zeros((P, NCT, G), f)
    gtmat = np.zeros((G, NCT, P), f)
    for j in range(NCT):
        grp = 8 * j + pidx // CPG
        gmat[pidx, j, grp] = 1.0
        gtmat[grp, j, pidx] = 1.0
    common["gmat"] = gmat
    common["gtmat"] = gtmat

    in_maps = []
    for b in range(B):
        m = dict(common)
        m["x"] = np.ascontiguousarray(x[b].reshape(NCT, P, T))
        in_maps.append(m)
    return in_maps


_NC_CACHE = []


def _get_nc():
    if not _NC_CACHE:
        _NC_CACHE.append(build_nc())
    return _NC_CACHE[0]


def run(in_maps, trace=False, **kw):
    from concourse.bass_utils import run_bass_kernel_spmd

    nc = _get_nc()
    return run_bass_kernel_spmd(nc, in_maps, list(range(B)), trace=trace, **kw)


def kernel(x, norm_w, norm_b, qkv_w, qkv_b, proj_w, proj_b):
    in_maps = prep_inputs(x, norm_w, norm_b, qkv_w, qkv_b, proj_w, proj_b)
    res = run(in_maps).results
    outs = [
        res[b]["out"].swapaxes(0, 1).reshape(C, 32, 32) for b in range(B)
    ]
    return np.stack(outs).astype(np.float32)


if __name__ == "__main__":
    nc = build_nc()
    print("built ok:", len(nc.m.functions[0].instructions) if hasattr(nc.m.functions[0], "instructions") else "n/a")



# revision 15
# speedup vs baseline: 8.8307x; 8.8307x over previous
"""Trainium2 Bass kernel for an AttentionBlock (GroupNorm + 8-head self-attn + proj + residual).

Sharding: data-parallel over batch. B=8 batch elements -> one per NeuronCore.
Each core runs an identical program on its own [C=512, T=1024] slice; the host
shards inputs / gathers outputs. No collectives.

Per-core pipeline (layouts partition-major, 128 partitions):
  x      [128p, 4ct, 1024t]   channels on partitions (fp32, kept for residual)
  GroupNorm stats: free-dim sum on VectorE + Square-with-accumulator on
     ScalarE; 16-channel group reduction and mean/rstd broadcast across
     partitions via tiny fp32 PE matmuls with 0/1 group matrices;
     rstd = exp(-0.5*ln(var+eps)); affine applied on ScalarE -> xn (bf16)
  qk   = Wqk @ xn + b  (bf16 matmuls, fp32 psum)  rows = [q(512) | k(512)],
         head-major 64-row blocks
  vT   = xn^T @ WvT    (bf16) stored per head-pair as [v_2j |1|1| v_2j+1]
         128-col blocks (ones columns make the AV matmul emit the softmax
         denominator Z replicated on the opposite partition half)
  lT[s,t] = k_h^T q_h per head: K=64 matmuls row-packed 2 heads per PE pass
  w    = exp(lT/8) on ScalarE, no max-subtraction (logits/8 in [-7,8]) -> bf16
  a|Z  = [v|1]^T @ w : 8-deep psum accumulation; per head the attention rows
         land on their final partition half, Z on the other
  1/Z  via DVE reciprocal_approx_fast at base partition 0 (custom-DVE ucode
         breaks at other bases); a 64-partition SBUF->SBUF DMA does the
         cross-partition move (engines cannot shift partitions)
  out  = WpT^T @ a * (1/Z) ... + (proj_b + Wp @ v_bias) + x  (v-bias folded on
         host; bias-add on ScalarE, residual on VectorE)

Schedule: one-pair software pipeline -- logits/exp of pair j+1 are emitted
before AV of pair j, with the remaining qk row-tiles spread across the pair
windows to keep the PE busy (HAM clock-throttle avoidance).
"""

import numpy as np

import concourse.bass as bass
import concourse.mybir as mybir
import concourse.tile as tile
from concourse import bacc
from contextlib import ExitStack

B = 8
C = 512
T = 1024
NH = 8            # heads
CH = 64           # channels per head
G = 32            # groups
CPG = C // G      # 16 channels per group
EPS = 1e-5
P = 128
NCT = C // P      # 4 channel tiles
NTT = T // P      # 8 sequence tiles
NQK = (2 * C) // P  # 8 row tiles of [q;k]
NC2 = T // 512    # 2 free-dim chunks of 512
HS = 2 * CH       # 128: per-head lhsT width in vT (64 v cols + 64 ones cols)

F32 = mybir.dt.float32
F32R = mybir.dt.float32r
BF16 = mybir.dt.bfloat16
FP8 = mybir.dt.float8e4
FX = mybir.ActivationFunctionType
ALU = mybir.AluOpType
AX = mybir.AxisListType


_DEBUG_TAP = None  # set by debug scripts before build_nc()
_STAGE = 5  # 1=GN/xn 2=+qkv/vT 3=+logits/exp 4=+AV 5=full (timing bisection)
_AV_MODE = "full"  # full | copy (no normalize) | local (no DMA, wrong nums)
_SMALL_BUFS = 2


def _build_body(ctx, tc, io):
    nc = tc.nc
    x_d = io["x"]
    out_d = io["out"]

    pers = ctx.enter_context(tc.tile_pool(name="pers", bufs=1))
    wt_pool = ctx.enter_context(tc.tile_pool(name="wt", bufs=3))
    small = ctx.enter_context(tc.tile_pool(name="small", bufs=_SMALL_BUFS))
    outp = ctx.enter_context(tc.tile_pool(name="outp", bufs=3))
    ps_a = ctx.enter_context(tc.tile_pool(name="ps_a", bufs=2, space="PSUM"))
    ps_b = ctx.enter_context(tc.tile_pool(name="ps_b", bufs=4, space="PSUM"))

    # ---- persistent SBUF tensors -------------------------------------------
    x_sb = pers.tile([P, NCT, T], F32, tag="x")
    xn_sb = pers.tile([P, NCT, T], BF16, tag="xn")
    wqkT_sb = pers.tile([P, NCT, 2 * C], BF16, tag="wqkT")
    wvT_sb = pers.tile([P, NCT, C], BF16, tag="wvT")
    wpT_sb = pers.tile([P, NCT, C], BF16, tag="wpT")
    qk_sb = pers.tile([P, NQK, T], BF16, tag="qk")
    vT_sb = pers.tile([P, NTT, NH * HS], BF16, tag="vT")
    a_sb = pers.tile([P, NCT, T], BF16, tag="a")
    gw_sb = pers.tile([P, NCT], F32, tag="gw")
    gb_sb = pers.tile([P, NCT], F32, tag="gb")
    bqk_sb = pers.tile([P, NQK], F32, tag="bqk")
    bp_sb = pers.tile([P, NCT], F32, tag="bp")
    gmat_sb = pers.tile([P, NCT, G], F32, tag="gmat")
    gtmat_sb = pers.tile([G, NCT, P], F32, tag="gtmat")
    stats_sb = pers.tile([P, NCT, 2], F32, tag="stats")
    stats2_sb = pers.tile([G, 2], F32, tag="stats2")
    ab_sb = pers.tile([P, NCT, 2], F32, tag="ab")
    xsq_sb = pers.tile([P, T], F32, tag="xsq")

    # ---- input DMAs ---------------------------------------------------------
    # x split per channel-tile (4 DMA queues in parallel; GN stats for tile j
    # start as soon as slice j lands)
    for j in range(NCT):
        nc.sync.dma_start(x_sb[:, j, :], x_d[j])
    nc.sync.dma_start(gmat_sb[:], io["gmat"])
    nc.sync.dma_start(gtmat_sb[:], io["gtmat"])
    nc.sync.dma_start(gw_sb[:], io["gw"])
    nc.sync.dma_start(gb_sb[:], io["gb"])
    nc.sync.dma_start(wqkT_sb[:], io["wqkT"])
    nc.sync.dma_start(bqk_sb[:], io["bqk"])
    nc.sync.dma_start(wvT_sb[:], io["wvT"])
    nc.sync.dma_start(wpT_sb[:], io["wpT"])
    nc.sync.dma_start(bp_sb[:], io["bp"])

    # Per head-pair j, vT cols j*256..(j+1)*256 hold [v_2j | ones | ones | v_2j+1].
    # The AV lhsT for head h is cols h*128..(h+1)*128: [v|1] for even heads,
    # [1|v] for odd heads, so each head's attention rows land on the same
    # partition half as its final slot in a_sb, and the softmax denominator
    # lands replicated on the other half -- every consumer op stays
    # partition-aligned (HW engines cannot shift data across partitions).
    ones_view = vT_sb[:, :, : NH * HS].rearrange(
        "p s (pr i) -> p s pr i", i=2 * HS
    )[:, :, :, CH : CH + HS]
    nc.vector.memset(ones_view, 1.0)

    # ---- GroupNorm statistics ----------------------------------------------
    # Per-channel sum and sum-of-squares on ScalarE: the activation's fused
    # accumulator reduces along the free dim for free, and ScalarE is
    # otherwise idle during the prologue.
    for j in range(NCT):
        nc.vector.reduce_sum(stats_sb[:, j, 0:1], x_sb[:, j, :], axis=AX.X)
        nc.scalar.activation(
            xsq_sb[:], x_sb[:, j, :], FX.Square, accum_out=stats_sb[:, j, 1:2]
        )
    gstat_ps = ps_b.tile([P, 512], F32, tag="psb")
    for j in range(NCT):
        nc.tensor.matmul(
            gstat_ps[0:G, 0:2],
            lhsT=gmat_sb[:, j, :],
            rhs=stats_sb[:, j, :],
            start=(j == 0),
            stop=(j == NCT - 1),
        )
    # finalize on 32 partitions: mean, var -> rstd (sqrt + recip + 1 Newton step)
    mv = small.tile([G, 2], F32, tag="mv")
    nc.vector.tensor_scalar_mul(mv[:], gstat_ps[0:G, 0:2], 1.0 / (CPG * T))
    msq = small.tile([G, 1], F32, tag="msq")
    nc.vector.tensor_mul(msq[:], mv[:, 0:1], mv[:, 0:1])
    veps = small.tile([G, 1], F32, tag="veps")
    nc.vector.tensor_scalar(veps[:], msq[:], -1.0, EPS, ALU.mult, ALU.add)
    nc.vector.tensor_tensor(veps[:], mv[:, 1:2], veps[:], ALU.add)  # E2-mean^2+eps
    # rstd = exp(-0.5 * ln(var+eps)); Ln and Exp share one ACT table set and
    # are far more accurate than the Sqrt table (~2 ULP vs 65536 budget)
    lnv = small.tile([G, 1], F32, tag="lnv")
    nc.scalar.activation(lnv[:], veps[:], FX.Ln)
    nc.scalar.activation(stats2_sb[:, 1:2], lnv[:], FX.Exp, scale=-0.5)
    nc.vector.tensor_copy(stats2_sb[:, 0:1], mv[:, 0:1])

    # broadcast (mean, rstd) to channels; A = w*rstd, B = b - mean*A; xn = A*x+B
    ab_ps = ps_b.tile([P, 512], F32, tag="psb")
    for j in range(NCT):
        nc.tensor.matmul(
            ab_ps[:, 2 * j : 2 * j + 2],
            lhsT=gtmat_sb[:, j, :],
            rhs=stats2_sb[:],
            start=True,
            stop=True,
            skip_group_check=True,
        )
    mean_v = ab_ps[:, 0:8].rearrange("p (j two) -> p j two", two=2)[:, :, 0]
    rstd_v = ab_ps[:, 0:8].rearrange("p (j two) -> p j two", two=2)[:, :, 1]
    a_v = ab_sb[:, :, 0]
    b_v = ab_sb[:, :, 1]
    nc.vector.tensor_tensor(a_v, gw_sb[:, :], rstd_v, ALU.mult)
    nc.vector.tensor_tensor(b_v, mean_v, a_v, ALU.mult)
    nc.vector.tensor_tensor(b_v, gb_sb[:, :], b_v, ALU.subtract)
    for j in range(NCT):
        nc.scalar.activation(
            xn_sb[:, j, :],
            x_sb[:, j, :],
            FX.Identity,
            bias=ab_sb[:, j, 1:2],
            scale=ab_sb[:, j, 0:1],
        )

    # ---- qk = Wqk @ xn + b --------------------------------------------------
    def emit_qk(m):
        for n in range(NC2):
            qk_ps = ps_b.tile([P, 512], F32, tag="psb")
            for k in range(NCT):
                nc.tensor.matmul(
                    qk_ps[:],
                    lhsT=wqkT_sb[:, k, m * P : (m + 1) * P],
                    rhs=xn_sb[:, k, n * 512 : (n + 1) * 512],
                    start=(k == 0),
                    stop=(k == NCT - 1),
                )
            nc.vector.tensor_scalar_add(
                qk_sb[:, m, n * 512 : (n + 1) * 512], qk_ps[:], bqk_sb[:, m : m + 1]
            )

    def emit_vt(st):
        vt_ps = ps_b.tile([P, 512], F32, tag="psb")
        for k in range(NCT):
            nc.tensor.matmul(
                vt_ps[:],
                lhsT=xn_sb[:, k, st * P : (st + 1) * P],
                rhs=wvT_sb[:, k, :],
                start=(k == 0),
                stop=(k == NCT - 1),
            )
        blk = vT_sb[:, st, : NH * HS].rearrange("p (pr i) -> p pr i", i=2 * HS)
        src = vt_ps[:].rearrange("p (pr half i) -> p pr half i", half=2, i=CH)
        nc.vector.tensor_copy(blk[:, :, 0:CH], src[:, :, 0, :])
        nc.vector.tensor_copy(blk[:, :, 2 * HS - CH : 2 * HS], src[:, :, 1, :])

    def emit_pair_logits(j, wt):
        """Logits + exp for head pair (2j, 2j+1); row-packed K=64 matmuls.
        Both halves of an s-tile share one 4-bank psum tile so the exp is a
        single FD=2048 ScalarE instruction (amortizes the per-op init)."""
        for st in range(NTT):
            for half in range(2):
                lt = ps_a.tile([P, T], F32, tag="psa")
                rs = slice(half * CH, (half + 1) * CH)
                for n in range(NC2):
                    nc.tensor.matmul(
                        lt[:, n * 512 : (n + 1) * 512],
                        lhsT=qk_sb[rs, 4 + j, st * P : (st + 1) * P],
                        rhs=qk_sb[rs, j, n * 512 : (n + 1) * 512],
                        start=True,
                        stop=True,
                    )
                nc.scalar.activation(
                    wt[:, st, half * T : (half + 1) * T], lt[:], FX.Exp, scale=0.125
                )

    def emit_pair_av(j, wt):
        for half in range(2):
            h = 2 * j + half
            for n in range(NC2):
                av_ps = ps_b.tile([P, 512], F32, tag="psb")
                for st in range(NTT):
                    nc.tensor.matmul(
                        av_ps[:],
                        lhsT=vT_sb[:, st, h * HS : (h + 1) * HS],
                        rhs=wt[:, st, half * T + n * 512 : half * T + (n + 1) * 512],
                        start=(st == 0),
                        stop=(st == NTT - 1),
                    )
                if _DEBUG_TAP == "av0" and j == 0:
                    dt_t = outp.tile([P, 512], F32, tag="dbg", name=f"dbg{h}_{n}")
                    nc.vector.tensor_copy(dt_t[:], av_ps[:])
                    nc.sync.dma_start(
                        out_d[:, half, n * 512 : (n + 1) * 512], dt_t[:]
                    )
                # even head: rows 0..63 = attention, 64..127 = Z (odd: swapped).
                # The custom-DVE reciprocal only works at base partition 0, and
                # engines can't shift data across partitions -- a small
                # SBUF->SBUF DMA does the 64-partition move.
                lo, hi = slice(0, CH), slice(CH, P)
                if _AV_MODE == "copy":
                    sl = lo if half == 0 else hi
                    nc.vector.tensor_copy(
                        a_sb[sl, j, n * 512 : (n + 1) * 512], av_ps[sl, :]
                    )
                    continue
                if _AV_MODE == "local":
                    # structurally mirrors "full" (copy -> link -> recip -> mult)
                    # but replaces the partition-shift DMA with a same-partition
                    # DVE copy; numerics are wrong for one half, timing only.
                    sl = lo if half == 0 else hi
                    zi = small.tile([P, 512], F32, tag="zi")
                    if half == 0:
                        zc = small.tile([P, 512], F32, tag="zc")
                        nc.vector.tensor_copy(zc[lo, :], av_ps[lo, :])
                        nc.vector.reciprocal_approx_fast(zi[lo, :], zc[lo, :])
                    else:
                        nc.vector.tensor_copy(zi[hi, :], av_ps[hi, :])
                    nc.vector.tensor_tensor(
                        a_sb[sl, j, n * 512 : (n + 1) * 512],
                        av_ps[sl, :],
                        zi[sl, :],
                        ALU.mult,
                    )
                    continue
                zs = small.tile([P, 512], F32, tag="zs")
                if half == 0:
                    zc = small.tile([P, 512], F32, tag="zc")
                    nc.vector.tensor_copy(zc[hi, :], av_ps[hi, :])
                    nc.sync.dma_start(zs[lo, :], zc[hi, :])
                    zi = small.tile([P, 512], F32, tag="zi")
                    nc.vector.reciprocal_approx_fast(zi[lo, :], zs[lo, :])
                    nc.vector.tensor_tensor(
                        a_sb[lo, j, n * 512 : (n + 1) * 512],
                        av_ps[lo, :],
                        zi[lo, :],
                        ALU.mult,
                    )
                else:
                    zi = small.tile([P, 512], F32, tag="zi")
                    nc.vector.reciprocal_approx_fast(zi[lo, :], av_ps[lo, :])
                    nc.sync.dma_start(zs[hi, :], zi[lo, :])
                    nc.vector.tensor_tensor(
                        a_sb[hi, j, n * 512 : (n + 1) * 512],
                        av_ps[hi, :],
                        zs[hi, :],
                        ALU.mult,
                    )

    def tap_out(src3d):
        for m in range(NCT):
            dt_t = outp.tile([P, T], F32, tag="dbg", name=f"stg{m}")
            nc.vector.tensor_copy(dt_t[:], src3d[:, m, :])
            nc.sync.dma_start(out_d[:, m, :], dt_t[:])

    if _STAGE == 1:
        tap_out(xn_sb)
        return

    # start attention on pair 0 as soon as its q/k row tiles exist
    emit_qk(0)
    emit_qk(4)
    wts0 = wt_pool.tile([P, NTT, 2 * T], BF16, tag="w", name="wt0")
    emit_pair_logits(0, wts0)
    if _DEBUG_TAP == "w0":
        for m in range(NCT):
            dt_t = outp.tile([P, T], F32, tag="dbg", name=f"dbg{m}")
            nc.vector.tensor_copy(dt_t[:], wts0[:, m, 0:T])
            nc.sync.dma_start(out_d[:, m, :], dt_t[:])
        return
    for st in range(NTT):
        emit_vt(st)
    if _DEBUG_TAP == "av0":
        emit_pair_av(0, wts0)
        return
    if _STAGE == 2:
        tap_out(qk_sb[:, 0:NCT, :])
        return
    if _STAGE == 3:
        # logits+exp only: tiny reads release the wt slots without AV
        dmy = small.tile([P, 8], F32, tag="dmy")
        nc.vector.tensor_copy(dmy[:, 0:8], wts0[:, 7, 2040:2048])
        for j in range(1, 4):
            wts = wt_pool.tile([P, NTT, 2 * T], BF16, tag="w", name=f"wt{j}")
            emit_pair_logits(j, wts)
            nc.vector.tensor_copy(dmy[:, 0:8], wts[:, 7, 2040:2048])
        tap_out(qk_sb[:, 0:NCT, :])
        return
    # One-pair software pipeline: logits(j+1) are emitted (= higher scheduler
    # priority) before AV(j), so ScalarE always has exp input ready while the
    # PE drains the previous pair's AV matmuls in the gaps. The remaining
    # qk row tiles are spread across the pair windows to keep the PE from
    # idling long enough to trip the HAM clock throttle.
    prev = wts0
    emit_qk(1)
    emit_qk(5)
    for j in range(1, 4):
        wts = wt_pool.tile([P, NTT, 2 * T], BF16, tag="w", name=f"wt{j}")
        emit_pair_logits(j, wts)
        emit_pair_av(j - 1, prev)
        if j < 3:
            emit_qk(j + 1)
            emit_qk(5 + j)
        prev = wts
    emit_pair_av(3, prev)
    if _STAGE == 4:
        tap_out(a_sb)
        return

    # optional debug tap: overwrite `out` with an intermediate tensor
    if _DEBUG_TAP == "xn":
        for m in range(NCT):
            dt_t = outp.tile([P, T], F32, tag="dbg", name=f"dbg{m}")
            nc.vector.tensor_copy(dt_t[:], xn_sb[:, m, :])
            nc.sync.dma_start(out_d[:, m, :], dt_t[:])
        return
    if _DEBUG_TAP == "qk":  # q rows: qk tiles 0..3 -> out tiles 0..3
        for m in range(NCT):
            dt_t = outp.tile([P, T], F32, tag="dbg", name=f"dbg{m}")
            nc.vector.tensor_copy(dt_t[:], qk_sb[:, m, :])
            nc.sync.dma_start(out_d[:, m, :], dt_t[:])
        return
    if _DEBUG_TAP == "k":  # k rows: qk tiles 4..7
        for m in range(NCT):
            dt_t = outp.tile([P, T], F32, tag="dbg", name=f"dbg{m}")
            nc.vector.tensor_copy(dt_t[:], qk_sb[:, 4 + m, :])
            nc.sync.dma_start(out_d[:, m, :], dt_t[:])
        return
    if _DEBUG_TAP == "vt":  # vT tiles 0..3 (with ones cols)
        for m in range(NCT):
            dt_t = outp.tile([P, T], F32, tag="dbg", name=f"dbg{m}")
            nc.vector.tensor_copy(dt_t[:], vT_sb[:, m, :])
            nc.sync.dma_start(out_d[:, m, :], dt_t[:])
        return
    if _DEBUG_TAP == "a":
        for m in range(NCT):
            dt_t = outp.tile([P, T], F32, tag="dbg", name=f"dbg{m}")
            nc.vector.tensor_copy(dt_t[:], a_sb[:, m, :])
            nc.sync.dma_start(out_d[:, m, :], dt_t[:])
        return

    # ---- proj + bias + residual --------------------------------------------
    for m in range(NCT):
        for n in range(NC2):
            pr_ps = ps_b.tile([P, 512], F32, tag="psb")
            for k in range(NCT):
                nc.tensor.matmul(
                    pr_ps[:],
                    lhsT=wpT_sb[:, k, m * P : (m + 1) * P],
                    rhs=a_sb[:, k, n * 512 : (n + 1) * 512],
                    start=(k == 0),
                    stop=(k == NCT - 1),
                )
            ot = outp.tile([P, 512], F32, tag="ot")
            # bias-add on ScalarE (idle in the tail), residual on VectorE
            nc.scalar.activation(
                ot[:], pr_ps[:], FX.Identity, bias=bp_sb[:, m : m + 1]
            )
            nc.vector.tensor_add(
                ot[:], ot[:], x_sb[:, m, n * 512 : (n + 1) * 512]
            )
            nc.sync.dma_start(out_d[:, m, n * 512 : (n + 1) * 512], ot[:])


def build_nc(loop_n=0):
    """loop_n > 0 wraps the body in a For_i running it loop_n times --
    used only by the timing harness to amortize host/RPC overhead."""
    nc = bacc.Bacc("TRN2", target_bir_lowering=False, debug=False)
    io = {}
    io["x"] = nc.dram_tensor("x", [NCT, P, T], F32, kind="ExternalInput").ap()
    io["gw"] = nc.dram_tensor("gw", [P, NCT], F32, kind="ExternalInput").ap()
    io["gb"] = nc.dram_tensor("gb", [P, NCT], F32, kind="ExternalInput").ap()
    io["wqkT"] = nc.dram_tensor("wqkT", [P, NCT, 2 * C], BF16, kind="ExternalInput").ap()
    io["bqk"] = nc.dram_tensor("bqk", [P, NQK], F32, kind="ExternalInput").ap()
    io["wvT"] = nc.dram_tensor("wvT", [P, NCT, C], BF16, kind="ExternalInput").ap()
    io["wpT"] = nc.dram_tensor("wpT", [P, NCT, C], BF16, kind="ExternalInput").ap()
    io["bp"] = nc.dram_tensor("bp", [P, NCT], F32, kind="ExternalInput").ap()
    io["gmat"] = nc.dram_tensor("gmat", [P, NCT, G], F32, kind="ExternalInput").ap()
    io["gtmat"] = nc.dram_tensor("gtmat", [G, NCT, P], F32, kind="ExternalInput").ap()
    io["out"] = nc.dram_tensor("out", [P, NCT, T], F32, kind="ExternalOutput").ap()
    with tile.TileContext(nc) as tc:
        with ExitStack() as ctx:
            if loop_n:
                with tc.For_i(0, loop_n, 1):
                    _build_body(ctx, tc, io)
            else:
                _build_body(ctx, tc, io)
    nc.compile()
    return nc


def _tile_cmaj(a, ntiles):
    """[ntiles*128, F...] -> [128, ntiles, F...] (partition-major tiling)."""
    return np.ascontiguousarray(
        a.reshape(ntiles, P, *a.shape[1:]).swapaxes(0, 1)
    )


def prep_inputs(x, norm_w, norm_b, qkv_w, qkv_b, proj_w, proj_b):
    f = np.float32
    x = np.asarray(x, f)
    norm_w = np.asarray(norm_w, f)
    norm_b = np.asarray(norm_b, f)
    qkv_w = np.asarray(qkv_w, f)
    qkv_b = np.asarray(qkv_b, f)
    proj_w = np.asarray(proj_w, f)
    proj_b = np.asarray(proj_b, f)

    wr = qkv_w.reshape(NH, 3, CH, C)
    Wq = wr[:, 0].reshape(C, C)
    Wk = wr[:, 1].reshape(C, C)
    Wv = wr[:, 2].reshape(C, C)
    br = qkv_b.reshape(NH, 3, CH)
    bq = br[:, 0].reshape(C)
    bk = br[:, 1].reshape(C)
    bv = br[:, 2].reshape(C)

    common = {}
    common["gw"] = _tile_cmaj(norm_w, NCT)
    common["gb"] = _tile_cmaj(norm_b, NCT)
    import ml_dtypes
    bf = ml_dtypes.bfloat16
    common["wqkT"] = _tile_cmaj(np.concatenate([Wq, Wk], 0).T.copy(), NCT).astype(bf)
    common["bqk"] = _tile_cmaj(np.concatenate([bq, bk]), NQK)
    common["wvT"] = _tile_cmaj(Wv.T.copy(), NCT).astype(bf)
    common["wpT"] = _tile_cmaj(proj_w.T.copy(), NCT).astype(bf)
    common["bp"] = _tile_cmaj(proj_b + proj_w @ bv, NCT)

    pidx = np.arange(P)
    gmat = np.zeros((P, NCT, G), f)
    gtmat = np.zeros((G, NCT, P), f)
    for j in range(NCT):
        grp = 8 * j + pidx // CPG
        gmat[pidx, j, grp] = 1.0
        gtmat[grp, j, pidx] = 1.0
    common["gmat"] = gmat
    common["gtmat"] = gtmat

    in_maps = []
    for b in range(B):
        m = dict(common)
        m["x"] = np.ascontiguousarray(x[b].reshape(NCT, P, T))
        in_maps.append(m)
    return in_maps


_NC_CACHE = []


def _get_nc():
    if not _NC_CACHE:
        _NC_CACHE.append(build_nc())
    return _NC_CACHE[0]


def run(in_maps, trace=False, **kw):
    from concourse.bass_utils import run_bass_kernel_spmd

    nc = _get_nc()
    return run_bass_kernel_spmd(nc, in_maps, list(range(B)), trace=trace, **kw)


def kernel(x, norm_w, norm_b, qkv_w, qkv_b, proj_w, proj_b):
    in_maps = prep_inputs(x, norm_w, norm_b, qkv_w, qkv_b, proj_w, proj_b)
    res = run(in_maps).results
    outs = [
        res[b]["out"].swapaxes(0, 1).reshape(C, 32, 32) for b in range(B)
    ]
    return np.stack(outs).astype(np.float32)


if __name__ == "__main__":
    nc = build_nc()
    print("built ok:", len(nc.m.functions[0].instructions) if hasattr(nc.m.functions[0], "instructions") else "n/a")

